# revision 1
# baseline (speedup 1.0000x reference)
"""ContraNorm Trainium2 kernel: out = 1.2*x - 0.2 * softmax(xn @ xn^T) @ x per batch.

Full input x [8, 2048, 512] f32; batch dim sharded across 8 NeuronCores
(data-parallel, no collectives). Each core runs an identical Bass/Tile program
on its [2048, 512] slice.

v3: exploits symmetry of S = xn @ xn^T. Per 256-col block M, MM1 computes
only row-chunks j >= 2M (144 of 256 [128,128] subtiles). Strictly-lower
subtiles (j >= 2M+2) are exp'd ONCE per wave to a uniform bf16 stage
[P, NT, 14, P]; their fp8 expST slots are one GPSIMD cast per wave, and
upper-triangle slots of later blocks come from PE transposes of the stage
(groups of 4 per PSUM bank) + ACT fp8 cast. Engine balance targets:
PE ~52us (125k col-slots at 1 col/cycle), ACT ~36us (exp + mirror casts),
DVE ~33us (stats, finals, xnT copies), Pool ~26us (xe/xn/direct casts).

Per-core pipeline (fp8 DoubleRow matmuls, fp32 PSUM accumulation):
  setup: load x; bn_stats per tile -> batched 1/sqrt (2 ACT-table loads
         total); xn = x * rn bf16 on GPSIMD; PE-transpose xn -> xnT fp8;
         xe fp8 = x + ones col (one GPSIMD copy).
  per block M: mirror transposes; MM1 waves; per-wave exp (fp8 direct for
  rows 2M..2M+1 / bf16 stage + GPSIMD fp8 cast for rows >= 2M+2);
  MM2 per half h: O = expST.T @ [x | 1] -> PSUM (ones col -> denom D);
  final: out = 1.2*x + (-0.2/D) * O on DVE; DMA out on the Act ring.
"""

import sys

if "/opt/trn_rl_repo" not in sys.path:
    sys.path.insert(0, "/opt/trn_rl_repo")

from contextlib import ExitStack

import numpy as np

import concourse.bass as bass
import concourse.tile as tile
import concourse.mybir as mybir
from concourse import bacc
from concourse.masks import make_identity
from concourse.bass_utils import run_bass_kernel_spmd

F32 = mybir.dt.float32
BF16 = mybir.dt.bfloat16
FP8 = mybir.dt.float8e4
AF = mybir.ActivationFunctionType
ALU = mybir.AluOpType

B = 8
P = 128
N = 2048
D = 512
NT = N // P      # 16 row tiles
DS = D // P      # 4 d subtiles
MB = 256         # m superblock
NBLK = N // MB   # 8
SK = 14          # stage slots per row-chunk (max k of strictly-lower tiles)


def contranorm_body(ctx: ExitStack, tc: tile.TileContext, out_ap: bass.AP, x_ap: bass.AP):
    nc = tc.nc

    singles = ctx.enter_context(tc.tile_pool(name="singles", bufs=1))
    scratch = ctx.enter_context(tc.tile_pool(name="scratch", bufs=3))
    stats = ctx.enter_context(tc.tile_pool(name="stats", bufs=8))
    xnpool = ctx.enter_context(tc.tile_pool(name="xnpool", bufs=4))

    xf = singles.tile([P, NT, D], F32)        # x, natural layout (n on partitions)
    xe = singles.tile([P, NT, D + 16], FP8)   # x + ones column at [.., D]
    xnT = singles.tile([P, DS, N], FP8)       # xn transposed (d on partitions)
    stage = singles.tile([P, NT, SK, P], BF16)  # exp'd strictly-lower subtiles
    mvAll = singles.tile([P, NT, 2], F32)     # bn_aggr outputs (mean, var)
    rnAll = singles.tile([P, NT], F32)        # 1/||x_row||
    ident = singles.tile([P, P], BF16)
    make_identity(nc, ident)
    nc.vector.memset(xe[:, :, D:D + 1], 1.0)

    # PSUM budget (8 banks): tpsum 2x1, psumS 2x2, psumO 1x2.
    tpsum = ctx.enter_context(tc.tile_pool(name="tpsum", bufs=2, space="PSUM"))
    psumS = ctx.enter_context(tc.tile_pool(name="psumS", bufs=2, space="PSUM"))
    psumO = ctx.enter_context(tc.tile_pool(name="psumO", bufs=1, space="PSUM"))

    # ---------------- setup: stats, batched norms, xn, transpose ----------------
    for i in range(NT):
        nc.sync.dma_start(xf[:, i, :], x_ap[i * P:(i + 1) * P, :])
        bst = scratch.tile([P, nc.vector.BN_STATS_DIM], F32, tag="bst")
        nc.vector.bn_stats(bst, xf[:, i, :])
        nc.vector.bn_aggr(mvAll[:, i, :], bst)

    # ssq/D = mean^2 + var; rn = 1/sqrt(ssq)  (norms ~22.6 >> eps: clamp is a no-op)
    vpm = stats.tile([P, NT], F32, tag="vpm")
    nc.vector.tensor_tensor(vpm, mvAll[:, :, 0], mvAll[:, :, 0], op=ALU.mult)
    nc.vector.tensor_add(vpm, vpm, mvAll[:, :, 1])
    nrm = stats.tile([P, NT], F32, tag="nrm")
    nc.scalar.activation(nrm, vpm, AF.Sqrt, scale=float(D))
    nc.vector.reciprocal(rnAll, nrm)

    nc.gpsimd.tensor_copy(xe[:, :, 0:D], xf)
    for i in range(NT):
        xn = xnpool.tile([P, D], BF16, tag="xn")
        nc.gpsimd.tensor_scalar_mul(xn, xf[:, i, :], rnAll[:, i:i + 1])
        pt = tpsum.tile([P, DS, P], BF16, tag="pt")
        for dc in range(DS):
            nc.tensor.transpose(pt[:, dc, :], xn[:, dc * P:(dc + 1) * P], ident)
        nc.vector.tensor_copy(xnT[:, :, i * P:(i + 1) * P], pt)

    # ---------------- main loop over 256-col blocks ----------------
    epool = ctx.enter_context(tc.tile_pool(name="epool", bufs=3))
    opool = ctx.enter_context(tc.tile_pool(name="opool", bufs=3))

    for M in range(NBLK):
        mlo = M * MB
        jlo0 = 2 * M           # first computed row-chunk
        nj = NT - jlo0
        expST = epool.tile([P, NT, MB], FP8, tag="expST")

        # mirrors: expST[:, k, :] for k < 2M by PE-transposing stage tiles
        # (groups of <=4 per PSUM bank), ACT fp8 cast into place
        for bi, b in enumerate((2 * M, 2 * M + 1)):
            nk = 2 * M
            for k0 in range(0, nk, 4):
                kch = min(4, nk - k0)
                mt = tpsum.tile([P, DS, P], BF16, tag="pt")
                for kk in range(kch):
                    nc.tensor.transpose(mt[:, kk, :], stage[:, b, k0 + kk, :], ident)
                nc.scalar.copy(
                    expST[:, k0:k0 + kch, bi * P:(bi + 1) * P], mt[:, 0:kch, :])

        # MM1 waves over computed row-chunks
        w = 0
        while w * 4 < nj:
            jlo = jlo0 + w * 4
            wch = min(4, NT - jlo)
            ps = psumS.tile([P, 4, MB], F32, tag="ps")
            for c in range(wch):
                j = jlo + c
                for g in range(DS // 2):
                    nc.tensor.matmul(
                        ps[:, c, :],
                        lhsT=xnT[:, 2 * g:2 * g + 2, j * P:(j + 1) * P],
                        rhs=xnT[:, 2 * g:2 * g + 2, mlo:mlo + MB],
                        start=(g == 0),
                        stop=(g == DS // 2 - 1),
                        perf_mode=mybir.MatmulPerfMode.DoubleRow,
                    )
            if w == 0:
                # rows 2M, 2M+1: diagonal/upper region -> direct fp8 exp
                nc.scalar.activation(expST[:, jlo:jlo + 2, :], ps[:, 0:2, :], AF.Exp)
            # strictly-lower rows (j >= 2M+2): exp once to bf16 stage
            # (batched per wave), fp8 direct slots via one GPSIMD cast
            j0s = max(jlo, jlo0 + 2)
            nr = jlo + wch - j0s
            if nr > 0:
                c0 = j0s - jlo
                st = stage[:, j0s:jlo + wch, 2 * M:2 * M + 2, :]
                nc.scalar.activation(
                    st, ps[:, c0:c0 + nr, :].rearrange("p w (a b) -> p w a b", a=2),
                    AF.Exp)
                nc.gpsimd.tensor_copy(
                    expST[:, j0s:jlo + wch, :].rearrange("p w (a b) -> p w a b", a=2),
                    st)
            w += 1

        # MM2 + finals for the two output row-tiles of this block
        for h in range(2):
            i = 2 * M + h
            po = psumO.tile([P, 1024], F32, tag="po")
            for g in range(NT // 2):
                lhsT = expST[:, 2 * g:2 * g + 2, h * P:(h + 1) * P]
                nc.tensor.matmul(po[:, 0:256], lhsT, xe[:, 2 * g:2 * g + 2, 0:256],
                                 start=(g == 0), stop=(g == NT // 2 - 1),
                                 perf_mode=mybir.MatmulPerfMode.DoubleRow)
                nc.tensor.matmul(po[:, 512:512 + 257], lhsT,
                                 xe[:, 2 * g:2 * g + 2, 256:D + 1],
                                 start=(g == 0), stop=(g == NT // 2 - 1),
                                 perf_mode=mybir.MatmulPerfMode.DoubleRow)
            # s = -0.2 / D  (D at psum col 768)
            sD = stats.tile([P, 1], F32, tag="sD")
            nc.vector.tensor_scalar_mul(sD, po[:, 768:769], -5.0)
            rD = stats.tile([P, 1], F32, tag="rD")
            nc.vector.reciprocal(rD, sD)
            tmp = opool.tile([P, 2, 256], F32, tag="tmp")
            po3 = po.rearrange("p (b c) -> p b c", b=2, c=512)[:, :, 0:256]
            nc.vector.tensor_scalar_mul(tmp, po3, rD)
            ob = opool.tile([P, D], F32, tag="ob")
            nc.vector.scalar_tensor_tensor(
                ob, xf[:, i, :], 1.2, tmp.rearrange("p b c -> p (b c)"),
                op0=ALU.mult, op1=ALU.add)
            nc.scalar.dma_start(out_ap[i * P:(i + 1) * P, :], ob)


def build_nc(repeats: int = 1, loop: int = 0):
    """Build + compile the per-core Bass program. `repeats` re-emits the body
    (sharing pools/SBUF); `loop` wraps the body in a For_i hardware loop --
    both are for steady-state timing measurements."""
    nc = bacc.Bacc("TRN2", target_bir_lowering=False, debug=False, enable_asserts=False)
    x = nc.dram_tensor("x", [N, D], F32, kind="ExternalInput").ap()
    out = nc.dram_tensor("out", [N, D], F32, kind="ExternalOutput").ap()
    with tile.TileContext(nc) as tc:
        if loop:
            with ExitStack() as ctx:
                with tc.For_i(0, loop, 1):
                    contranorm_body(ctx, tc, out, x)
        else:
            for _ in range(repeats):
                with ExitStack() as ctx:
                    contranorm_body(ctx, tc, out, x)
    nc.compile()
    return nc


_nc_cache = {}


def kernel(x: np.ndarray) -> np.ndarray:
    assert x.shape == (B, N, D), x.shape
    x = np.ascontiguousarray(x, dtype=np.float32)
    if "nc" not in _nc_cache:
        _nc_cache["nc"] = build_nc()
    nc = _nc_cache["nc"]
    in_maps = [{"x": x[i]} for i in range(B)]
    res = run_bass_kernel_spmd(nc, in_maps, core_ids=list(range(B)))
    return np.stack([r["out"] for r in res.results], axis=0)

